# revision 3
# baseline (speedup 1.0000x reference)
"""LinFormer ragged-segment kernel for 8 Trainium2 NeuronCores (SPMD).

Math (per batch b, per segment s of the ragged sequence):
    k = elu(f @ Wk + bk) + 1      q = elu(f @ Wq + bq) + 1      v = f @ Wv
    ktv[s]  = k[s]^T @ v[s]                       # [K, O] per segment
    out[s]  = q[s] @ ktv[s]                       # [L_s, O]

Sharding: the 64 (batch, segment) tasks are paired into 8 uniform "slots"
so that all 8 cores run an identical static program on differently-packed
data. Cores 0-3 take batch 0-3 with the larger member of each slot pair,
cores 4-7 take batch 0-3 with the smaller member (zero-padded; padding
tokens have v == 0 exactly, so they contribute nothing to ktv, and their
outputs are dropped at gather time).

Device dataflow per 512-token block (all matmul operands bf16, fp32 PSUM):
    qk^T [128,512] = [Wq|Wk]^T @ f^T      (weights stationary, f^T moving)
    v^T  [128,512] = Wv^T @ f^T
    elu+1 via: k = max(x,0) + min(exp(x),1)   (exact identity)
    k chunks DMA-transposed to token-on-partition; ktv accumulated in PSUM
    out^T [128,512] = ktv^T @ q^T          (one matmul per block)
The kernel returns out^T per core; the host transposes and gathers.
"""

import sys

import numpy as np

for _p in ("/opt/trn_rl_repo",):
    if _p not in sys.path:
        sys.path.insert(0, _p)

import ml_dtypes

import concourse.bass as bass
import concourse.mybir as mybir
import concourse.tile as tile
from concourse import bacc
from concourse.bass_utils import run_bass_kernel_spmd

# ---------------------------------------------------------------- constants
B, N, C_IN, K_DIM, Q_DIM, C_OUT = 4, 100000, 256, 64, 64, 128
LENGTHS = np.array(
    [3000, 5000, 7000, 2000, 9000, 4000, 6000, 8000,
     1000, 10000, 5500, 6500, 7500, 4500, 9500, 11500],
    dtype=np.int64,
)
OFFS = np.concatenate([[0], np.cumsum(LENGTHS)]).astype(np.int64)

# slot template: blocks of 512 tokens per slot; identical on every core
T_J = [23, 19, 16, 14, 12, 10, 8, 4]           # blocks per slot
SEG_BIG = [15, 14, 7, 2, 6, 1, 5, 3]           # cores 0-3 (exact fit)
SEG_SMALL = [9, 4, 12, 11, 10, 13, 0, 8]       # cores 4-7 (zero-padded)
BLK = 512
N_BLOCKS = sum(T_J)                            # 106
T_TOK = N_BLOCKS * BLK                         # 54272
SLOT_OFF = np.concatenate([[0], np.cumsum([t * BLK for t in T_J])]).astype(int)

N_CORES = 8
BF16 = mybir.dt.bfloat16
F32 = mybir.dt.float32

_compiled = None  # (nc, has_bias) cache


def _build(has_bias: bool):
    nc = bacc.Bacc("TRN2", target_bir_lowering=False, debug=False,
                   num_devices=N_CORES)

    ft = nc.dram_tensor("ft", [C_IN, T_TOK], BF16, kind="ExternalInput").ap()
    wqk = nc.dram_tensor("wqk", [2, 128, 128], BF16, kind="ExternalInput").ap()
    wv = nc.dram_tensor("wv", [2, 128, 128], BF16, kind="ExternalInput").ap()
    if has_bias:
        bqk = nc.dram_tensor("bqk", [128, 1], F32, kind="ExternalInput").ap()
    out = nc.dram_tensor("out", [C_OUT, T_TOK], F32, kind="ExternalOutput").ap()

    ft_r = ft.rearrange("(h p) t -> p h t", h=2)
    wqk_r = wqk.rearrange("h p m -> p h m")
    wv_r = wv.rearrange("h p m -> p h m")

    Exp = mybir.ActivationFunctionType.Exp
    amax = mybir.AluOpType.max
    aadd = mybir.AluOpType.add

    from contextlib import ExitStack
    with tile.TileContext(nc, trace_sim=False) as tc, ExitStack() as ctx:
        wpool = ctx.enter_context(tc.tile_pool(name="w", bufs=1))
        sb = ctx.enter_context(tc.tile_pool(name="sb", bufs=3))
        kqpool = ctx.enter_context(tc.tile_pool(name="kqs", bufs=28))
        ps = ctx.enter_context(tc.tile_pool(name="ps", bufs=2, space="PSUM"))

        w_qk = wpool.tile([128, 2, 128], BF16, tag="wqk")
        nc.sync.dma_start(out=w_qk[:], in_=wqk_r[:])
        w_v = wpool.tile([128, 2, 128], BF16, tag="wv")
        nc.sync.dma_start(out=w_v[:], in_=wv_r[:])
        if has_bias:
            b_qk = wpool.tile([128, 1], F32, tag="bqk")
            nc.sync.dma_start(out=b_qk[:], in_=bqk[:])

        for j, tj in enumerate(T_J):
            ktv_ps = ps.tile([64, 128], F32, tag="ktv")
            kq_tiles = []
            for bi in range(tj):
                blk = SLOT_OFF[j] // BLK + bi
                tsl = slice(blk * BLK, (blk + 1) * BLK)

                ft_sb = sb.tile([128, 2, BLK], BF16, tag="ft")
                nc.sync.dma_start(out=ft_sb[:], in_=ft_r[:, :, tsl])

                kq_ps = ps.tile([128, BLK], F32, tag="kqps")
                nc.tensor.matmul(kq_ps[:], lhsT=w_qk[:, 0, :], rhs=ft_sb[:, 0, :],
                                 start=True, stop=False)
                nc.tensor.matmul(kq_ps[:], lhsT=w_qk[:, 1, :], rhs=ft_sb[:, 1, :],
                                 start=False, stop=True)
                vt_ps = ps.tile([128, BLK], F32, tag="vtps")
                nc.tensor.matmul(vt_ps[:], lhsT=w_v[:, 0, :], rhs=ft_sb[:, 0, :],
                                 start=True, stop=False)
                nc.tensor.matmul(vt_ps[:], lhsT=w_v[:, 1, :], rhs=ft_sb[:, 1, :],
                                 start=False, stop=True)

                # elu(x)+1 == max(x,0) + min(exp(x),1), exact
                e_sb = sb.tile([128, BLK], F32, tag="e")
                if has_bias:
                    nc.scalar.activation(e_sb[:], kq_ps[:], Exp, bias=b_qk[:, 0:1])
                else:
                    nc.scalar.activation(e_sb[:], kq_ps[:], Exp)
                t_sb = sb.tile([128, BLK], F32, tag="t")
                nc.vector.tensor_scalar_min(t_sb[:], e_sb[:], 1.0)
                kq_sb = kqpool.tile([128, BLK], BF16, tag="kq")
                if has_bias:
                    r_sb = sb.tile([128, BLK], F32, tag="r")
                    nc.vector.tensor_scalar(r_sb[:], kq_ps[:], b_qk[:, 0:1], 0.0,
                                            aadd, amax)
                    nc.vector.tensor_add(kq_sb[:], r_sb[:], t_sb[:])
                else:
                    nc.vector.scalar_tensor_tensor(kq_sb[:], in0=kq_ps[:],
                                                   scalar=0.0, in1=t_sb[:],
                                                   op0=amax, op1=aadd)
                kq_tiles.append(kq_sb)

                vt_sb = sb.tile([128, BLK], BF16, tag="vt")
                nc.scalar.copy(vt_sb[:], vt_ps[:])

                for c in range(BLK // 128):
                    csl = slice(c * 128, (c + 1) * 128)
                    k_t = sb.tile([128, 64], BF16, tag="k_t")
                    nc.sync.dma_start_transpose(k_t[:], kq_sb[64:128, csl])
                    v_t = sb.tile([128, 128], BF16, tag="v_t")
                    nc.sync.dma_start_transpose(v_t[:], vt_sb[:, csl])
                    nc.tensor.matmul(ktv_ps[:], lhsT=k_t[:], rhs=v_t[:],
                                     start=(bi == 0 and c == 0),
                                     stop=(bi == tj - 1 and c == BLK // 128 - 1))

            ktv_sb = sb.tile([64, 128], BF16, tag="ktvs")
            nc.scalar.copy(ktv_sb[:], ktv_ps[:])

            for bi in range(tj):
                blk = SLOT_OFF[j] // BLK + bi
                tsl = slice(blk * BLK, (blk + 1) * BLK)
                ot_ps = ps.tile([128, BLK], F32, tag="otps")
                nc.tensor.matmul(ot_ps[:], lhsT=ktv_sb[:],
                                 rhs=kq_tiles[bi][0:64, :],
                                 start=True, stop=True)
                ot_sb = sb.tile([128, BLK], F32, tag="ot")
                nc.scalar.copy(ot_sb[:], ot_ps[:])
                nc.sync.dma_start(out=out[:, tsl], in_=ot_sb[:])

    nc.compile()
    return nc


def _get_compiled(has_bias: bool):
    global _compiled
    if _compiled is None or _compiled[1] != has_bias:
        _compiled = (_build(has_bias), has_bias)
    return _compiled[0]


def _prep_inputs(features, Wk, bk, Wq, bq, Wv):
    """Build per-core input maps (host-side shard + transpose + cast)."""
    bf16 = ml_dtypes.bfloat16
    wqk_full = np.concatenate([Wq, Wk], axis=1)          # [256, 128] q|k
    wqk = np.stack([wqk_full[0:128], wqk_full[128:256]]).astype(bf16)
    wv = np.stack([Wv[0:128], Wv[128:256]]).astype(bf16)
    has_bias = bool(np.any(bk) or np.any(bq))
    bqk = np.concatenate([bq, bk]).astype(np.float32).reshape(128, 1)

    in_maps = []
    for c in range(N_CORES):
        b = c % 4
        segs = SEG_BIG if c < 4 else SEG_SMALL
        ftc = np.zeros((C_IN, T_TOK), dtype=bf16)
        for j, seg in enumerate(segs):
            s, e = int(OFFS[seg]), int(OFFS[seg + 1])
            off = int(SLOT_OFF[j])
            ftc[:, off:off + (e - s)] = features[b, s:e, :].astype(bf16).T
        m = {"ft": ftc, "wqk": wqk, "wv": wv}
        if has_bias:
            m["bqk"] = bqk
        in_maps.append(m)
    return in_maps, has_bias


def _gather(results):
    """Assemble full [B, N, C_OUT] fp32 output from per-core out^T tensors."""
    out = np.empty((B, N, C_OUT), dtype=np.float32)
    for c in range(N_CORES):
        b = c % 4
        segs = SEG_BIG if c < 4 else SEG_SMALL
        ot = results[c]["out"]                            # [128, T_TOK]
        for j, seg in enumerate(segs):
            s, e = int(OFFS[seg]), int(OFFS[seg + 1])
            off = int(SLOT_OFF[j])
            out[b, s:e, :] = ot[:, off:off + (e - s)].T
    return out


def kernel(features, lengths, Wk, bk, Wq, bq, Wv):
    features = np.asarray(features, dtype=np.float32)
    lengths = np.asarray(lengths)
    Wk = np.asarray(Wk, dtype=np.float32)
    bk = np.asarray(bk, dtype=np.float32)
    Wq = np.asarray(Wq, dtype=np.float32)
    bq = np.asarray(bq, dtype=np.float32)
    Wv = np.asarray(Wv, dtype=np.float32)
    assert np.array_equal(np.asarray(lengths, dtype=np.int64), LENGTHS), \
        "kernel compiled for the fixed ragged layout of this problem"

    in_maps, has_bias = _prep_inputs(features, Wk, bk, Wq, bq, Wv)
    nc = _get_compiled(has_bias)
    res = run_bass_kernel_spmd(nc, in_maps, list(range(N_CORES)))
    return _gather(res.results)


if __name__ == "__main__":
    rng = np.random.default_rng(0)
    feats = rng.standard_normal((B, N, C_IN), dtype=np.float32)
    Wk_ = rng.standard_normal((C_IN, K_DIM), dtype=np.float32) / 16
    Wq_ = rng.standard_normal((C_IN, Q_DIM), dtype=np.float32) / 16
    Wv_ = rng.standard_normal((C_IN, C_OUT), dtype=np.float32) / 16
    o = kernel(feats, LENGTHS, Wk_, np.zeros(K_DIM, np.float32),
               Wq_, np.zeros(Q_DIM, np.float32), Wv_)
    print("kernel ran, out shape", o.shape)
